# revision 8
# baseline (speedup 1.0000x reference)
"""Trainium2 Bass kernel for nn_HadamardExpansionV2 (topk_masking).

Sharding: data-parallel over batch B=16 across 8 cores (2 samples/core);
weights replicated. CrossHadaNorm batch stats via AllReduce of per-channel
sum/sumsq.

Host precompute folds BN into the conv: W2 = diag(s) @ fc_w, b2 = (fc_b -
bn_mean)*s + bn_beta with s = gamma/sqrt(var+eps), so y = W2 @ x + b2.
Logits fold: E = (eva_w @ W2)/1024, f = eva_w @ b2 + eva_b, so
logits = E @ sum_hw(x) + f (computed exact fp32 on device: min top-32
boundary gap is ~1e-4, so the logits path cannot use f32r).

Per-core schedule:
  - x/weight chunks load on sync-DGE; E^T/f/consts load via scalar-DGE in
    parallel; per-chunk tiles so consumers start as chunks land.
  - PE: conv m0 -> logits -> conv m1..m3 (covers the top-k wait) ->
    Wsel transposes -> selconv -> hadamard pa/pb.
  - DVE: xbar row-sums early, top-k rounds, prod multiplies, s1 reduces,
    post-AllReduce affine.  ACT: conv epilogues, pa copies, Square+accum
    for s2.  A dummy AllReduce at t=0 warms the collective stream.
  - AllReduce of [s1|s2] -> affine -> streamed prod writes.
"""

import os
import sys

import numpy as np

for _p in ("/opt/trn_rl_repo", os.path.expanduser("~/.axon_site/_ro/trn_rl_repo")):
    if os.path.isdir(_p) and _p not in sys.path:
        sys.path.insert(0, _p)

import concourse.bass as bass
import concourse.mybir as mybir
import concourse.tile as tile
from concourse import bacc
from concourse.bass_utils import run_bass_kernel_spmd

C1 = 512
CS = 32
CSE = 496  # 32*31/2
HWD = 1024  # H*W
B = 16
NCORES = 8
SPC = B // NCORES  # samples per core
P = 128
KC = C1 // P  # 4 contraction chunks
MC = C1 // P  # 4 output-channel chunks
NF = 512  # matmul free dim (PSUM bank)
NNC = HWD // NF  # 2 free chunks
EPS = 1e-5
NTOT = float(B * HWD)

HI, HJ = np.triu_indices(CS, k=1)

F32 = mybir.dt.float32
F32R = mybir.dt.float32r
U32 = mybir.dt.uint32

# M-chunking of the 496 expanded channels: 128,128,128,112
EXP_M = [(0, 128), (128, 128), (256, 128), (384, 112)]

USE_TTR = os.environ.get("USE_TTR", "0") == "1"
WARM_CC = os.environ.get("WARM_CC", "1") == "1"


def build_program():
    nc = bacc.Bacc(
        "TRN2",
        target_bir_lowering=False,
        debug=False,
        num_devices=NCORES,
    )

    # ---------------- I/O ----------------
    xs = nc.dram_tensor("xs", [SPC, C1, HWD], F32R, kind="ExternalInput")
    w2T = nc.dram_tensor("w2T", [C1, C1], F32R, kind="ExternalInput")  # [c, o]
    w2rows = nc.dram_tensor("w2rows", [C1, C1], F32, kind="ExternalInput")  # [o, c]
    eT = nc.dram_tensor("eT", [C1, C1], F32, kind="ExternalInput")  # [c, o']
    f2 = nc.dram_tensor("f2", [SPC, C1], F32, kind="ExternalInput")
    b2vec = nc.dram_tensor("b2vec", [C1], F32, kind="ExternalInput")
    gpairp = nc.dram_tensor("gpairp", [C1], F32, kind="ExternalInput")
    bpairp = nc.dram_tensor("bpairp", [C1], F32, kind="ExternalInput")
    g_hi = nc.dram_tensor("g_hi", [CS, CSE], F32R, kind="ExternalInput")
    g_hj = nc.dram_tensor("g_hj", [CS, CSE], F32R, kind="ExternalInput")
    ident32 = nc.dram_tensor("ident32", [CS, CS], F32, kind="ExternalInput")

    outs = [
        nc.dram_tensor(f"out{s}", [C1 + CSE, HWD], F32, kind="ExternalOutput")
        for s in range(SPC)
    ]

    with tile.TileContext(nc) as tc:
        with (
            tc.tile_pool(name="const", bufs=1) as cpool,
            tc.tile_pool(name="xp", bufs=1) as xpool,
            tc.tile_pool(name="prodp", bufs=1) as prodpool,
            tc.tile_pool(name="yp", bufs=3) as ypool,
            tc.tile_pool(name="acp", bufs=4) as apool,
            tc.tile_pool(name="small", bufs=3) as spool,
            tc.tile_pool(name="psA", bufs=4, space="PSUM") as psA,
            tc.tile_pool(name="psB", bufs=4, space="PSUM") as psB,
            tc.tile_pool(name="dram", bufs=1, space="DRAM") as dpool,
        ):
            # ---- persistent SBUF tiles (per-chunk for dep granularity) ----
            w_k = [cpool.tile([P, C1], F32R, tag=f"w{k}", name=f"w{k}") for k in range(KC)]
            x_k = [
                [xpool.tile([P, HWD], F32R, tag=f"x{s}_{k}", name=f"x{s}_{k}") for k in range(KC)]
                for s in range(SPC)
            ]
            eT_sb = cpool.tile([P, KC, C1], F32, tag="eT", name="eT")
            f2_sb = cpool.tile([SPC, C1], F32, tag="f2", name="f2")
            b2v = cpool.tile([P, MC], F32, tag="b2v", name="b2v")
            gpair_v = cpool.tile([P, MC], F32, tag="gpv", name="gpv")
            bpair_v = cpool.tile([P, MC], F32, tag="bpv", name="bpv")
            i32_sb = cpool.tile([CS, CS], F32, tag="i32", name="i32")
            ghi_sb = cpool.tile([CS, CSE], F32R, tag="ghi", name="ghi")
            ghj_sb = cpool.tile([CS, CSE], F32R, tag="ghj", name="ghj")
            xbar2 = cpool.tile([P, KC, SPC], F32, tag="xbar", name="xbar")
            stats = cpool.tile([P, 2 * MC], F32, tag="stats", name="stats")
            s1p = cpool.tile([P, MC, NNC * SPC], F32, tag="s1p", name="s1p")
            s2p = cpool.tile([P, MC, NNC * SPC], F32, tag="s2p", name="s2p")
            gstats = cpool.tile([P, 2 * MC], F32, tag="gstats", name="gstats")
            eps_col = cpool.tile([P, 1], F32, tag="eps", name="eps")
            prod_sb = [
                prodpool.tile([P, MC, HWD], F32, tag=f"prod{s}", name=f"prod{s}")
                for s in range(SPC)
            ]

            # ---- collective warm-up (dummy AllReduce on stream 0) ----
            if WARM_CC:
                ccw_in = dpool.tile([1, 8], F32)
                ccw_out = dpool.tile([1, 8], F32)
                warm_src = cpool.tile([1, 8], F32, tag="warm", name="warm")
                nc.gpsimd.memset(warm_src[:], 0.0)
                nc.sync.dma_start(ccw_in[:], warm_src[:])
                nc.gpsimd.collective_compute(
                    "AllReduce",
                    mybir.AluOpType.add,
                    replica_groups=[list(range(NCORES))],
                    ins=[ccw_in.opt()],
                    outs=[ccw_out.opt()],
                )

            # ---- input DMAs: conv weights + x interleaved per k (sync DGE) ----
            for k in range(KC):
                nc.sync.dma_start(w_k[k][:], w2T.ap()[k * P : (k + 1) * P, :])
                for s in range(SPC):
                    nc.sync.dma_start(x_k[s][k][:], xs.ap()[s][k * P : (k + 1) * P, :])
            # consts via scalar DGE (parallel queue)
            nc.scalar.dma_start(eT_sb[:], eT.ap().rearrange("(ko p) o -> p ko o", p=P))
            nc.scalar.dma_start(f2_sb[:], f2.ap())
            nc.scalar.dma_start(b2v[:], b2vec.ap().rearrange("(m p) -> p m", p=P))
            nc.scalar.dma_start(gpair_v[:], gpairp.ap().rearrange("(m p) -> p m", p=P))
            nc.scalar.dma_start(bpair_v[:], bpairp.ap().rearrange("(m p) -> p m", p=P))
            nc.scalar.dma_start(i32_sb[:], ident32.ap())
            nc.scalar.dma_start(ghi_sb[:], g_hi.ap())
            nc.scalar.dma_start(ghj_sb[:], g_hj.ap())

            # ---- Pool: memsets;  DVE: early xbar row-sums ----
            nc.gpsimd.memset(stats[:], 0.0)
            nc.gpsimd.memset(s1p[:], 0.0)
            nc.gpsimd.memset(s2p[:], 0.0)
            nc.gpsimd.memset(eps_col[:], EPS)
            for k in range(KC):
                for s in range(SPC):
                    nc.vector.tensor_reduce(
                        xbar2[:, k, s : s + 1],
                        x_k[s][k].bitcast(F32)[:],
                        axis=mybir.AxisListType.X,
                        op=mybir.AluOpType.add,
                    )

            # ---- conv helper: one m-chunk, both samples ----
            def conv_mgroup(m):
                pts = {}
                for s in range(SPC):
                    for n in range(NNC):
                        pts[(s, n)] = psA.tile([P, NF], F32, tag="cv", name=f"cv{m}_{s}_{n}")
                for k in range(KC):
                    for s in range(SPC):
                        for n in range(NNC):
                            nc.tensor.matmul(
                                pts[(s, n)][:],
                                lhsT=w_k[k][:, m * P : (m + 1) * P],
                                rhs=x_k[s][k][:, n * NF : (n + 1) * NF],
                                start=(k == 0),
                                stop=(k == KC - 1),
                            )
                for s in range(SPC):
                    ych = ypool.tile([P, HWD], F32, tag="y", name=f"y{m}_{s}")
                    for n in range(NNC):
                        nc.scalar.activation(
                            ych[:, n * NF : (n + 1) * NF],
                            pts[(s, n)][:],
                            mybir.ActivationFunctionType.Identity,
                            bias=b2v[:, m : m + 1],
                            scale=1.0,
                        )
                    nc.sync.dma_start(
                        outs[s].ap()[m * P : (m + 1) * P, :], ych[:]
                    )

            conv_mgroup(0)

            # ---- logits: exact f32 ----
            pl = psB.tile([P, NF], F32, tag="pa", name="pl")
            for k in range(KC):
                nc.tensor.matmul(
                    pl[:SPC, :],
                    lhsT=xbar2[:, k, :],
                    rhs=eT_sb[:, k, :],
                    start=(k == 0),
                    stop=(k == KC - 1),
                )
            logit = spool.tile([SPC, C1], F32, tag="logit", name="logit")
            nc.vector.tensor_tensor(
                logit[:], pl[:SPC, :], f2_sb[:], op=mybir.AluOpType.add
            )

            # ---- top-32 (DVE) + idx roundtrip — emitted early so the idx
            # DMAs sit ahead of later y-write triggers on the sync queue ----
            idx_all = spool.tile([SPC, CS], U32, tag="idx", name="idx")
            for r in range(4):
                mx8 = spool.tile([SPC, 8], F32, tag="mx8", name=f"mx8_{r}")
                nc.vector.max(out=mx8[:], in_=logit[:])
                nc.vector.max_index(
                    out=idx_all[:, r * 8 : (r + 1) * 8], in_max=mx8[:], in_values=logit[:]
                )
                if r < 3:
                    nc.vector.match_replace(
                        out=logit[:], in_to_replace=mx8[:], in_values=logit[:],
                        imm_value=-1e30,
                    )
            idx_dram = dpool.tile([SPC, CS], U32)
            nc.sync.dma_start(idx_dram[:], idx_all[:])
            idx_col = [None] * SPC
            for s in range(SPC):
                idx_col[s] = spool.tile([CS, 1], U32, tag=f"idxc{s}", name=f"idxc{s}")
                nc.sync.dma_start(
                    idx_col[s][:],
                    idx_dram[s].rearrange("(p one) -> p one", one=1),
                )
            # gathers (gpsimd stream; fire when idx lands)
            wsel = [None] * SPC
            b2sel = [None] * SPC
            for s in range(SPC):
                wsel[s] = spool.tile([CS, C1], F32, tag=f"wsel{s}", name=f"wsel{s}")
                nc.gpsimd.indirect_dma_start(
                    out=wsel[s][:],
                    out_offset=None,
                    in_=w2rows.ap()[0:C1, :],
                    in_offset=bass.IndirectOffsetOnAxis(ap=idx_col[s][:, :1], axis=0),
                )
                b2sel[s] = spool.tile([CS, 1], F32, tag=f"b2s{s}", name=f"b2s{s}")
                nc.gpsimd.indirect_dma_start(
                    out=b2sel[s][:],
                    out_offset=None,
                    in_=b2vec.ap().rearrange("(c one) -> c one", one=1),
                    in_offset=bass.IndirectOffsetOnAxis(ap=idx_col[s][:, :1], axis=0),
                )

            # ---- conv m1..m3 (PE covers the top-k wait) ----
            conv_mgroup(1)
            conv_mgroup(2)
            conv_mgroup(3)

            # ---- selconv: xsel = W2[idx] @ x + b2[idx] ----
            xsel = [None] * SPC
            for s in range(SPC):
                ptr = psB.tile([P, NF], F32, tag="pa", name=f"ptr{s}")
                for k in range(KC):
                    nc.tensor.transpose(
                        ptr[:, k * CS : (k + 1) * CS],
                        wsel[s][:, k * P : (k + 1) * P],
                        i32_sb[:],
                    )
                wselT = spool.tile([P, KC, CS], F32R, tag=f"wT{s}", name=f"wselT{s}")
                nc.scalar.activation(
                    wselT[:], ptr[:, 0 : KC * CS], mybir.ActivationFunctionType.Copy
                )
                xsel[s] = spool.tile([CS, HWD], F32R, tag=f"xsel{s}", name=f"xsel{s}")
                for n in range(NNC):
                    psel = psA.tile([P, NF], F32, tag="cv", name=f"psel{s}_{n}")
                    for k in range(KC):
                        nc.tensor.matmul(
                            psel[:CS, :],
                            lhsT=wselT[:, k, :],
                            rhs=x_k[s][k][:, n * NF : (n + 1) * NF],
                            start=(k == 0),
                            stop=(k == KC - 1),
                        )
                    nc.scalar.activation(
                        xsel[s][:, n * NF : (n + 1) * NF],
                        psel[:CS, :],
                        mybir.ActivationFunctionType.Identity,
                        bias=b2sel[s][:, 0:1],
                        scale=1.0,
                    )

            # ---- hadamard expansion + batch stats ----
            for m, (mo, mw) in enumerate(EXP_M):
                pas = {}
                for s in range(SPC):
                    for n in range(NNC):
                        pas[(s, n)] = psB.tile([P, NF], F32, tag="pa", name=f"pa{m}_{s}_{n}")
                        nc.tensor.matmul(
                            pas[(s, n)][:mw, :],
                            lhsT=ghi_sb[:, mo : mo + mw],
                            rhs=xsel[s][:, n * NF : (n + 1) * NF],
                            start=True,
                            stop=True,
                        )
                pbs = {}
                for s in range(SPC):
                    for n in range(NNC):
                        pbs[(s, n)] = psA.tile([P, NF], F32, tag="cv", name=f"pb{m}_{s}_{n}")
                        nc.tensor.matmul(
                            pbs[(s, n)][:mw, :],
                            lhsT=ghj_sb[:, mo : mo + mw],
                            rhs=xsel[s][:, n * NF : (n + 1) * NF],
                            start=True,
                            stop=True,
                        )
                for s in range(SPC):
                    for n in range(NNC):
                        a_sb = apool.tile([P, NF], F32, tag="ac", name=f"ac{m}_{s}_{n}")
                        nc.scalar.activation(
                            a_sb[:mw, :], pas[(s, n)][:mw, :],
                            mybir.ActivationFunctionType.Copy,
                        )
                        pchunk = prod_sb[s][:, m, n * NF : (n + 1) * NF]
                        if USE_TTR:
                            nc.vector.tensor_tensor_reduce(
                                out=pchunk[:mw],
                                in0=a_sb[:mw, :],
                                in1=pbs[(s, n)][:mw, :],
                                scale=1.0,
                                scalar=stats[:mw, m : m + 1],
                                op0=mybir.AluOpType.mult,
                                op1=mybir.AluOpType.add,
                                accum_out=stats[:mw, m : m + 1],
                            )
                        else:
                            nc.vector.tensor_tensor(
                                pchunk[:mw], a_sb[:mw, :], pbs[(s, n)][:mw, :],
                                op=mybir.AluOpType.mult,
                            )
                            nc.vector.tensor_reduce(
                                s1p[:mw, m, n * SPC + s : n * SPC + s + 1],
                                pchunk[:mw],
                                axis=mybir.AxisListType.X,
                                op=mybir.AluOpType.add,
                            )
                        sq = apool.tile([P, NF], F32, tag="sq", name=f"sq{m}_{s}_{n}")
                        nc.scalar.activation(
                            sq[:mw, :],
                            pchunk[:mw],
                            mybir.ActivationFunctionType.Square,
                            accum_out=s2p[:mw, m, n * SPC + s : n * SPC + s + 1],
                        )

            # slots -> stats cols
            if not USE_TTR:
                nc.vector.tensor_reduce(
                    stats[:, 0:MC], s1p[:], axis=mybir.AxisListType.X,
                    op=mybir.AluOpType.add,
                )
            nc.vector.tensor_reduce(
                stats[:, MC : 2 * MC], s2p[:], axis=mybir.AxisListType.X,
                op=mybir.AluOpType.add,
            )

            # ---- cross-core AllReduce of [s1|s2] ----
            cc_in = dpool.tile([P, 2 * MC], F32)
            cc_out = dpool.tile([P, 2 * MC], F32)
            nc.sync.dma_start(cc_in[:], stats[:])
            nc.gpsimd.collective_compute(
                "AllReduce",
                mybir.AluOpType.add,
                replica_groups=[list(range(NCORES))],
                ins=[cc_in.opt()],
                outs=[cc_out.opt()],
            )
            nc.sync.dma_start(gstats[:], cc_out[:])

            # mean/var -> alpha = rstd*gpair ; beta2 = bpair - mean*alpha
            meanc = spool.tile([P, MC], F32, tag="meanc", name="meanc")
            nc.vector.tensor_scalar_mul(meanc[:], gstats[:, 0:MC], 1.0 / NTOT)
            varc = spool.tile([P, MC], F32, tag="varc", name="varc")
            nc.vector.tensor_scalar_mul(varc[:], gstats[:, MC : 2 * MC], 1.0 / NTOT)
            msq = spool.tile([P, MC], F32, tag="msq", name="msq")
            nc.vector.tensor_mul(msq[:], meanc[:], meanc[:])
            nc.vector.tensor_sub(varc[:], varc[:], msq[:])
            nc.scalar.activation(
                varc[:], varc[:], mybir.ActivationFunctionType.Sqrt,
                bias=eps_col[:, 0:1],
            )
            rstd = spool.tile([P, MC], F32, tag="rstd", name="rstd")
            nc.vector.reciprocal(rstd[:], varc[:])
            alpha = spool.tile([P, MC], F32, tag="alpha", name="alpha")
            nc.vector.tensor_mul(alpha[:], rstd[:], gpair_v[:])
            beta2 = spool.tile([P, MC], F32, tag="beta2", name="beta2")
            nc.vector.tensor_mul(beta2[:], meanc[:], alpha[:])
            nc.vector.tensor_sub(beta2[:], bpair_v[:], beta2[:])

            # ---- normalize + write prod rows (per (s, m), full 1024 cols) ----
            for s in range(SPC):
                for m, (mo, mw) in enumerate(EXP_M):
                    pch = prod_sb[s][:, m, :]
                    nc.vector.tensor_scalar(
                        pch[:mw],
                        pch[:mw],
                        alpha[:mw, m : m + 1],
                        beta2[:mw, m : m + 1],
                        op0=mybir.AluOpType.mult,
                        op1=mybir.AluOpType.add,
                    )
                    nc.sync.dma_start(
                        outs[s].ap()[C1 + mo : C1 + mo + mw, :], pch[:mw]
                    )

    nc.compile()
    return nc


_NC_CACHE = {}


def _get_program():
    if "nc" not in _NC_CACHE:
        _NC_CACHE["nc"] = build_program()
    return _NC_CACHE["nc"]


def _make_consts():
    ghi = np.zeros((CS, CSE), np.float32)
    ghj = np.zeros((CS, CSE), np.float32)
    ghi[HI, np.arange(CSE)] = 1.0
    ghj[HJ, np.arange(CSE)] = 1.0
    return ghi, ghj


def make_shared_inputs(fc_w, fc_b, bn_gamma, bn_beta, bn_mean, bn_var, eva_w, eva_b):
    g64 = np.asarray(bn_gamma, np.float64)
    s64 = g64 / np.sqrt(np.asarray(bn_var, np.float64) + EPS)
    W2_64 = s64[:, None] * np.asarray(fc_w, np.float64)
    b2_64 = (np.asarray(fc_b, np.float64) - np.asarray(bn_mean, np.float64)) * s64 \
        + np.asarray(bn_beta, np.float64)
    E64 = (np.asarray(eva_w, np.float64) @ W2_64) / float(HWD)
    f64 = np.asarray(eva_w, np.float64) @ b2_64 + np.asarray(eva_b, np.float64)

    W2 = W2_64.astype(np.float32)
    gam = np.asarray(bn_gamma, np.float32)
    bet = np.asarray(bn_beta, np.float32)
    gpair = np.zeros(C1, np.float32)
    bpair = np.zeros(C1, np.float32)
    gpair[:CSE] = gam[HI] * gam[HJ]
    bpair[:CSE] = bet[HI] * bet[HJ]
    ghi, ghj = _make_consts()
    return dict(
        w2T=np.ascontiguousarray(W2.T),
        w2rows=np.ascontiguousarray(W2),
        eT=np.ascontiguousarray(E64.astype(np.float32).T),
        f2=np.broadcast_to(f64.astype(np.float32), (SPC, C1)).copy(),
        b2vec=b2_64.astype(np.float32),
        gpairp=gpair,
        bpairp=bpair,
        g_hi=ghi,
        g_hj=ghj,
        ident32=np.eye(CS, dtype=np.float32),
    )


def make_in_maps(inputs):
    x = np.asarray(inputs["x"], np.float32).reshape(B, C1, HWD)
    shared = make_shared_inputs(
        inputs["fc_w"], inputs["fc_b"], inputs["bn_gamma"], inputs["bn_beta"],
        inputs["bn_mean"], inputs["bn_var"], inputs["eva_w"], inputs["eva_b"],
    )
    return [
        dict(shared, xs=np.ascontiguousarray(x[i * SPC : (i + 1) * SPC]))
        for i in range(NCORES)
    ]


def kernel(x, fc_w, fc_b, bn_gamma, bn_beta, bn_mean, bn_var, eva_w, eva_b):
    in_maps = make_in_maps(dict(
        x=x, fc_w=fc_w, fc_b=fc_b, bn_gamma=bn_gamma, bn_beta=bn_beta,
        bn_mean=bn_mean, bn_var=bn_var, eva_w=eva_w, eva_b=eva_b,
    ))
    nc = _get_program()
    res = run_bass_kernel_spmd(nc, in_maps, list(range(NCORES))).results
    out = np.empty((B, C1 + CSE, HWD), np.float32)
    for i in range(NCORES):
        for s in range(SPC):
            out[i * SPC + s] = res[i][f"out{s}"]
    return out.reshape(B, C1 + CSE, 32, 32)


# revision 9
# speedup vs baseline: 1.0047x; 1.0047x over previous
"""Trainium2 Bass kernel for nn_HadamardExpansionV2 (topk_masking).

Sharding: data-parallel over batch B=16 across 8 cores (2 samples/core);
weights replicated. CrossHadaNorm batch stats via AllReduce of per-channel
sum/sumsq.

Host precompute folds BN into the conv: W2 = diag(s) @ fc_w, b2 = (fc_b -
bn_mean)*s + bn_beta with s = gamma/sqrt(var+eps), so y = W2 @ x + b2.
Logits fold: E = (eva_w @ W2)/1024, f = eva_w @ b2 + eva_b, so
logits = E @ sum_hw(x) + f (computed exact fp32 on device: min top-32
boundary gap is ~1e-4, so the logits path cannot use f32r).

Per-core schedule:
  - x/weight chunks load on sync-DGE; E^T/f/consts load via scalar-DGE in
    parallel; per-chunk tiles so consumers start as chunks land.
  - PE: conv m0 -> logits -> conv m1..m3 (covers the top-k wait) ->
    Wsel transposes -> selconv -> hadamard pa/pb.
  - DVE: xbar row-sums early, top-k rounds, prod multiplies, s1 reduces,
    post-AllReduce affine.  ACT: conv epilogues, pa copies, Square+accum
    for s2.  A dummy AllReduce at t=0 warms the collective stream.
  - AllReduce of [s1|s2] -> affine -> streamed prod writes.
"""

import os
import sys

import numpy as np

for _p in ("/opt/trn_rl_repo", os.path.expanduser("~/.axon_site/_ro/trn_rl_repo")):
    if os.path.isdir(_p) and _p not in sys.path:
        sys.path.insert(0, _p)

import concourse.bass as bass
import concourse.mybir as mybir
import concourse.tile as tile
from concourse import bacc
from concourse.bass_utils import run_bass_kernel_spmd

C1 = 512
CS = 32
CSE = 496  # 32*31/2
HWD = 1024  # H*W
B = 16
NCORES = 8
SPC = B // NCORES  # samples per core
P = 128
KC = C1 // P  # 4 contraction chunks
MC = C1 // P  # 4 output-channel chunks
NF = 512  # matmul free dim (PSUM bank)
NNC = HWD // NF  # 2 free chunks
EPS = 1e-5
NTOT = float(B * HWD)

HI, HJ = np.triu_indices(CS, k=1)

F32 = mybir.dt.float32
F32R = mybir.dt.float32r
U32 = mybir.dt.uint32

# M-chunking of the 496 expanded channels: 128,128,128,112
EXP_M = [(0, 128), (128, 128), (256, 128), (384, 112)]

USE_TTR = os.environ.get("USE_TTR", "0") == "1"
WARM_CC = os.environ.get("WARM_CC", "1") == "1"


def build_program():
    nc = bacc.Bacc(
        "TRN2",
        target_bir_lowering=False,
        debug=False,
        num_devices=NCORES,
    )

    # ---------------- I/O ----------------
    xs = nc.dram_tensor("xs", [SPC, C1, HWD], F32R, kind="ExternalInput")
    w2T = nc.dram_tensor("w2T", [C1, C1], F32R, kind="ExternalInput")  # [c, o]
    w2rows = nc.dram_tensor("w2rows", [C1, C1], F32, kind="ExternalInput")  # [o, c]
    eT = nc.dram_tensor("eT", [C1, C1], F32, kind="ExternalInput")  # [c, o']
    f2 = nc.dram_tensor("f2", [SPC, C1], F32, kind="ExternalInput")
    b2vec = nc.dram_tensor("b2vec", [C1], F32, kind="ExternalInput")
    gpairp = nc.dram_tensor("gpairp", [C1], F32, kind="ExternalInput")
    bpairp = nc.dram_tensor("bpairp", [C1], F32, kind="ExternalInput")
    g_hi = nc.dram_tensor("g_hi", [CS, CSE], F32R, kind="ExternalInput")
    g_hj = nc.dram_tensor("g_hj", [CS, CSE], F32R, kind="ExternalInput")
    ident32 = nc.dram_tensor("ident32", [CS, CS], F32, kind="ExternalInput")
    ident128 = nc.dram_tensor("ident128", [P, P], F32, kind="ExternalInput")

    outs = [
        nc.dram_tensor(f"out{s}", [C1 + CSE, HWD], F32, kind="ExternalOutput")
        for s in range(SPC)
    ]

    with tile.TileContext(nc) as tc:
        with (
            tc.tile_pool(name="const", bufs=1) as cpool,
            tc.tile_pool(name="xp", bufs=1) as xpool,
            tc.tile_pool(name="prodp", bufs=1) as prodpool,
            tc.tile_pool(name="yp", bufs=3) as ypool,
            tc.tile_pool(name="acp", bufs=4) as apool,
            tc.tile_pool(name="small", bufs=3) as spool,
            tc.tile_pool(name="psA", bufs=4, space="PSUM") as psA,
            tc.tile_pool(name="psB", bufs=4, space="PSUM") as psB,
            tc.tile_pool(name="dram", bufs=1, space="DRAM") as dpool,
        ):
            # ---- persistent SBUF tiles (per-chunk for dep granularity) ----
            w_k = [cpool.tile([P, C1], F32R, tag=f"w{k}", name=f"w{k}") for k in range(KC)]
            x_k = [
                [xpool.tile([P, HWD], F32R, tag=f"x{s}_{k}", name=f"x{s}_{k}") for k in range(KC)]
                for s in range(SPC)
            ]
            e_k = [cpool.tile([P, C1], F32, tag=f"e{k}", name=f"e{k}") for k in range(KC)]
            f2_sb = cpool.tile([SPC, C1], F32, tag="f2", name="f2")
            b2v = cpool.tile([P, MC], F32, tag="b2v", name="b2v")
            gpair_v = cpool.tile([P, MC], F32, tag="gpv", name="gpv")
            bpair_v = cpool.tile([P, MC], F32, tag="bpv", name="bpv")
            i32_sb = cpool.tile([CS, CS], F32, tag="i32", name="i32")
            ident_sb = cpool.tile([P, P], F32, tag="i128", name="i128")
            ghi_sb = cpool.tile([CS, CSE], F32R, tag="ghi", name="ghi")
            ghj_sb = cpool.tile([CS, CSE], F32R, tag="ghj", name="ghj")
            xbar2 = cpool.tile([P, KC, SPC], F32, tag="xbar", name="xbar")
            stats = cpool.tile([P, 2 * MC], F32, tag="stats", name="stats")
            s1p = cpool.tile([P, MC, NNC * SPC], F32, tag="s1p", name="s1p")
            s2p = cpool.tile([P, MC, NNC * SPC], F32, tag="s2p", name="s2p")
            gstats = cpool.tile([P, 2 * MC], F32, tag="gstats", name="gstats")
            eps_col = cpool.tile([P, 1], F32, tag="eps", name="eps")
            prod_sb = [
                prodpool.tile([P, MC, HWD], F32, tag=f"prod{s}", name=f"prod{s}")
                for s in range(SPC)
            ]

            # ---- collective warm-up: trigger-only dummy AllReduce on stream 0
            # (reads uninitialized DRAM; result unused; no data dependencies)
            if WARM_CC:
                ccw_in = dpool.tile([1, 8], F32)
                ccw_out = dpool.tile([1, 8], F32)
                nc.gpsimd.collective_compute(
                    "AllReduce",
                    mybir.AluOpType.add,
                    replica_groups=[list(range(NCORES))],
                    ins=[ccw_in.opt()],
                    outs=[ccw_out.opt()],
                )

            # ---- input DMAs: conv weights + x interleaved per k (sync DGE) ----
            for k in range(KC):
                nc.sync.dma_start(w_k[k][:], w2T.ap()[k * P : (k + 1) * P, :])
                for s in range(SPC):
                    nc.sync.dma_start(x_k[s][k][:], xs.ap()[s][k * P : (k + 1) * P, :])
            # consts via scalar DGE (parallel queue)
            for k in range(KC):
                nc.scalar.dma_start(e_k[k][:], eT.ap()[k * P : (k + 1) * P, :])
            nc.scalar.dma_start(f2_sb[:], f2.ap())
            nc.scalar.dma_start(b2v[:], b2vec.ap().rearrange("(m p) -> p m", p=P))
            nc.scalar.dma_start(gpair_v[:], gpairp.ap().rearrange("(m p) -> p m", p=P))
            nc.scalar.dma_start(bpair_v[:], bpairp.ap().rearrange("(m p) -> p m", p=P))
            nc.scalar.dma_start(i32_sb[:], ident32.ap())
            nc.scalar.dma_start(ident_sb[:], ident128.ap())
            nc.scalar.dma_start(ghi_sb[:], g_hi.ap())
            nc.scalar.dma_start(ghj_sb[:], g_hj.ap())

            # ---- Pool: memsets;  DVE: early xbar row-sums ----
            nc.gpsimd.memset(stats[:], 0.0)
            nc.gpsimd.memset(s1p[:], 0.0)
            nc.gpsimd.memset(s2p[:], 0.0)
            nc.gpsimd.memset(eps_col[:], EPS)
            for k in range(KC):
                for s in range(SPC):
                    nc.vector.tensor_reduce(
                        xbar2[:, k, s : s + 1],
                        x_k[s][k].bitcast(F32)[:],
                        axis=mybir.AxisListType.X,
                        op=mybir.AluOpType.add,
                    )

            # ---- conv helper: one m-chunk, both samples ----
            def conv_mgroup(m):
                pts = {}
                for s in range(SPC):
                    for n in range(NNC):
                        pts[(s, n)] = psA.tile([P, NF], F32, tag="cv", name=f"cv{m}_{s}_{n}")
                for k in range(KC):
                    for s in range(SPC):
                        for n in range(NNC):
                            nc.tensor.matmul(
                                pts[(s, n)][:],
                                lhsT=w_k[k][:, m * P : (m + 1) * P],
                                rhs=x_k[s][k][:, n * NF : (n + 1) * NF],
                                start=(k == 0),
                                stop=(k == KC - 1),
                            )
                for s in range(SPC):
                    ych = ypool.tile([P, HWD], F32, tag="y", name=f"y{m}_{s}")
                    for n in range(NNC):
                        nc.scalar.activation(
                            ych[:, n * NF : (n + 1) * NF],
                            pts[(s, n)][:],
                            mybir.ActivationFunctionType.Identity,
                            bias=b2v[:, m : m + 1],
                            scale=1.0,
                        )
                    nc.sync.dma_start(
                        outs[s].ap()[m * P : (m + 1) * P, :], ych[:]
                    )

            conv_mgroup(0)

            # ---- logits: exact f32, column form (cheap on PE) ----
            # lcol[o_chunk, (m, s)] = sum_c E^T[c, o] * xbar[c, s]
            plc = psB.tile([P, NF], F32, tag="pa", name="plc")
            for m in range(MC):
                for k in range(KC):
                    nc.tensor.matmul(
                        plc[:, m * SPC : (m + 1) * SPC],
                        lhsT=e_k[k][:, m * P : (m + 1) * P],
                        rhs=xbar2[:, k, :],
                        start=(k == 0),
                        stop=(k == KC - 1),
                    )
            lcol = spool.tile([P, MC * SPC], F32, tag="lcol", name="lcol")
            nc.scalar.activation(
                lcol[:], plc[:, 0 : MC * SPC], mybir.ActivationFunctionType.Copy
            )
            pl = psB.tile([P, NF], F32, tag="pa", name="pl")
            for m in range(MC):
                nc.tensor.transpose(
                    pl[:SPC, m * P : (m + 1) * P],
                    lcol[:, m * SPC : (m + 1) * SPC],
                    ident_sb[:],
                )
            logit = spool.tile([SPC, C1], F32, tag="logit", name="logit")
            nc.vector.tensor_tensor(
                logit[:], pl[:SPC, :], f2_sb[:], op=mybir.AluOpType.add
            )

            # ---- top-32 (DVE) + idx roundtrip — emitted early so the idx
            # DMAs sit ahead of later y-write triggers on the sync queue ----
            idx_all = spool.tile([SPC, CS], U32, tag="idx", name="idx")
            for r in range(4):
                mx8 = spool.tile([SPC, 8], F32, tag="mx8", name=f"mx8_{r}")
                nc.vector.max(out=mx8[:], in_=logit[:])
                nc.vector.max_index(
                    out=idx_all[:, r * 8 : (r + 1) * 8], in_max=mx8[:], in_values=logit[:]
                )
                if r < 3:
                    nc.vector.match_replace(
                        out=logit[:], in_to_replace=mx8[:], in_values=logit[:],
                        imm_value=-1e30,
                    )
            idx_dram = dpool.tile([SPC, CS], U32)
            nc.sync.dma_start(idx_dram[:], idx_all[:])
            idx_col = [None] * SPC
            for s in range(SPC):
                idx_col[s] = spool.tile([CS, 1], U32, tag=f"idxc{s}", name=f"idxc{s}")
                nc.sync.dma_start(
                    idx_col[s][:],
                    idx_dram[s].rearrange("(p one) -> p one", one=1),
                )
            # gathers (gpsimd stream; fire when idx lands)
            wsel = [None] * SPC
            b2sel = [None] * SPC
            for s in range(SPC):
                wsel[s] = spool.tile([CS, C1], F32, tag=f"wsel{s}", name=f"wsel{s}")
                nc.gpsimd.indirect_dma_start(
                    out=wsel[s][:],
                    out_offset=None,
                    in_=w2rows.ap()[0:C1, :],
                    in_offset=bass.IndirectOffsetOnAxis(ap=idx_col[s][:, :1], axis=0),
                )
                b2sel[s] = spool.tile([CS, 1], F32, tag=f"b2s{s}", name=f"b2s{s}")
                nc.gpsimd.indirect_dma_start(
                    out=b2sel[s][:],
                    out_offset=None,
                    in_=b2vec.ap().rearrange("(c one) -> c one", one=1),
                    in_offset=bass.IndirectOffsetOnAxis(ap=idx_col[s][:, :1], axis=0),
                )

            # ---- conv m1..m3 (PE covers the top-k wait) ----
            conv_mgroup(1)
            conv_mgroup(2)
            conv_mgroup(3)

            # ---- selconv: xsel = W2[idx] @ x + b2[idx] ----
            xsel = [None] * SPC
            for s in range(SPC):
                ptr = psB.tile([P, NF], F32, tag="pa", name=f"ptr{s}")
                for k in range(KC):
                    nc.tensor.transpose(
                        ptr[:, k * CS : (k + 1) * CS],
                        wsel[s][:, k * P : (k + 1) * P],
                        i32_sb[:],
                    )
                wselT = spool.tile([P, KC, CS], F32R, tag=f"wT{s}", name=f"wselT{s}")
                nc.scalar.activation(
                    wselT[:], ptr[:, 0 : KC * CS], mybir.ActivationFunctionType.Copy
                )
                xsel[s] = spool.tile([CS, HWD], F32R, tag=f"xsel{s}", name=f"xsel{s}")
                for n in range(NNC):
                    psel = psA.tile([P, NF], F32, tag="cv", name=f"psel{s}_{n}")
                    for k in range(KC):
                        nc.tensor.matmul(
                            psel[:CS, :],
                            lhsT=wselT[:, k, :],
                            rhs=x_k[s][k][:, n * NF : (n + 1) * NF],
                            start=(k == 0),
                            stop=(k == KC - 1),
                        )
                    nc.scalar.activation(
                        xsel[s][:, n * NF : (n + 1) * NF],
                        psel[:CS, :],
                        mybir.ActivationFunctionType.Identity,
                        bias=b2sel[s][:, 0:1],
                        scale=1.0,
                    )

            # ---- hadamard expansion + batch stats ----
            for m, (mo, mw) in enumerate(EXP_M):
                pas = {}
                for s in range(SPC):
                    for n in range(NNC):
                        pas[(s, n)] = psB.tile([P, NF], F32, tag="pa", name=f"pa{m}_{s}_{n}")
                        nc.tensor.matmul(
                            pas[(s, n)][:mw, :],
                            lhsT=ghi_sb[:, mo : mo + mw],
                            rhs=xsel[s][:, n * NF : (n + 1) * NF],
                            start=True,
                            stop=True,
                        )
                pbs = {}
                for s in range(SPC):
                    for n in range(NNC):
                        pbs[(s, n)] = psA.tile([P, NF], F32, tag="cv", name=f"pb{m}_{s}_{n}")
                        nc.tensor.matmul(
                            pbs[(s, n)][:mw, :],
                            lhsT=ghj_sb[:, mo : mo + mw],
                            rhs=xsel[s][:, n * NF : (n + 1) * NF],
                            start=True,
                            stop=True,
                        )
                for s in range(SPC):
                    for n in range(NNC):
                        a_sb = apool.tile([P, NF], F32, tag="ac", name=f"ac{m}_{s}_{n}")
                        nc.scalar.activation(
                            a_sb[:mw, :], pas[(s, n)][:mw, :],
                            mybir.ActivationFunctionType.Copy,
                        )
                        pchunk = prod_sb[s][:, m, n * NF : (n + 1) * NF]
                        if USE_TTR:
                            nc.vector.tensor_tensor_reduce(
                                out=pchunk[:mw],
                                in0=a_sb[:mw, :],
                                in1=pbs[(s, n)][:mw, :],
                                scale=1.0,
                                scalar=stats[:mw, m : m + 1],
                                op0=mybir.AluOpType.mult,
                                op1=mybir.AluOpType.add,
                                accum_out=stats[:mw, m : m + 1],
                            )
                        else:
                            nc.vector.tensor_tensor(
                                pchunk[:mw], a_sb[:mw, :], pbs[(s, n)][:mw, :],
                                op=mybir.AluOpType.mult,
                            )
                            nc.vector.tensor_reduce(
                                s1p[:mw, m, n * SPC + s : n * SPC + s + 1],
                                pchunk[:mw],
                                axis=mybir.AxisListType.X,
                                op=mybir.AluOpType.add,
                            )
                        sq = apool.tile([P, NF], F32, tag="sq", name=f"sq{m}_{s}_{n}")
                        nc.scalar.activation(
                            sq[:mw, :],
                            pchunk[:mw],
                            mybir.ActivationFunctionType.Square,
                            accum_out=s2p[:mw, m, n * SPC + s : n * SPC + s + 1],
                        )

            # slots -> stats cols
            if not USE_TTR:
                nc.vector.tensor_reduce(
                    stats[:, 0:MC], s1p[:], axis=mybir.AxisListType.X,
                    op=mybir.AluOpType.add,
                )
            nc.vector.tensor_reduce(
                stats[:, MC : 2 * MC], s2p[:], axis=mybir.AxisListType.X,
                op=mybir.AluOpType.add,
            )

            # ---- cross-core AllReduce of [s1|s2] ----
            cc_in = dpool.tile([P, 2 * MC], F32)
            cc_out = dpool.tile([P, 2 * MC], F32)
            nc.sync.dma_start(cc_in[:], stats[:])
            nc.gpsimd.collective_compute(
                "AllReduce",
                mybir.AluOpType.add,
                replica_groups=[list(range(NCORES))],
                ins=[cc_in.opt()],
                outs=[cc_out.opt()],
            )
            nc.sync.dma_start(gstats[:], cc_out[:])

            # mean/var -> alpha = rstd*gpair ; beta2 = bpair - mean*alpha
            meanc = spool.tile([P, MC], F32, tag="meanc", name="meanc")
            nc.vector.tensor_scalar_mul(meanc[:], gstats[:, 0:MC], 1.0 / NTOT)
            varc = spool.tile([P, MC], F32, tag="varc", name="varc")
            nc.vector.tensor_scalar_mul(varc[:], gstats[:, MC : 2 * MC], 1.0 / NTOT)
            msq = spool.tile([P, MC], F32, tag="msq", name="msq")
            nc.vector.tensor_mul(msq[:], meanc[:], meanc[:])
            nc.vector.tensor_sub(varc[:], varc[:], msq[:])
            nc.scalar.activation(
                varc[:], varc[:], mybir.ActivationFunctionType.Sqrt,
                bias=eps_col[:, 0:1],
            )
            rstd = spool.tile([P, MC], F32, tag="rstd", name="rstd")
            nc.vector.reciprocal(rstd[:], varc[:])
            alpha = spool.tile([P, MC], F32, tag="alpha", name="alpha")
            nc.vector.tensor_mul(alpha[:], rstd[:], gpair_v[:])
            beta2 = spool.tile([P, MC], F32, tag="beta2", name="beta2")
            nc.vector.tensor_mul(beta2[:], meanc[:], alpha[:])
            nc.vector.tensor_sub(beta2[:], bpair_v[:], beta2[:])

            # ---- normalize + write prod rows (per (s, m), full 1024 cols) ----
            for s in range(SPC):
                for m, (mo, mw) in enumerate(EXP_M):
                    pch = prod_sb[s][:, m, :]
                    nc.vector.tensor_scalar(
                        pch[:mw],
                        pch[:mw],
                        alpha[:mw, m : m + 1],
                        beta2[:mw, m : m + 1],
                        op0=mybir.AluOpType.mult,
                        op1=mybir.AluOpType.add,
                    )
                    nc.sync.dma_start(
                        outs[s].ap()[C1 + mo : C1 + mo + mw, :], pch[:mw]
                    )

    nc.compile()
    return nc


_NC_CACHE = {}


def _get_program():
    if "nc" not in _NC_CACHE:
        _NC_CACHE["nc"] = build_program()
    return _NC_CACHE["nc"]


def _make_consts():
    ghi = np.zeros((CS, CSE), np.float32)
    ghj = np.zeros((CS, CSE), np.float32)
    ghi[HI, np.arange(CSE)] = 1.0
    ghj[HJ, np.arange(CSE)] = 1.0
    return ghi, ghj


def make_shared_inputs(fc_w, fc_b, bn_gamma, bn_beta, bn_mean, bn_var, eva_w, eva_b):
    g64 = np.asarray(bn_gamma, np.float64)
    s64 = g64 / np.sqrt(np.asarray(bn_var, np.float64) + EPS)
    W2_64 = s64[:, None] * np.asarray(fc_w, np.float64)
    b2_64 = (np.asarray(fc_b, np.float64) - np.asarray(bn_mean, np.float64)) * s64 \
        + np.asarray(bn_beta, np.float64)
    E64 = (np.asarray(eva_w, np.float64) @ W2_64) / float(HWD)
    f64 = np.asarray(eva_w, np.float64) @ b2_64 + np.asarray(eva_b, np.float64)

    W2 = W2_64.astype(np.float32)
    gam = np.asarray(bn_gamma, np.float32)
    bet = np.asarray(bn_beta, np.float32)
    gpair = np.zeros(C1, np.float32)
    bpair = np.zeros(C1, np.float32)
    gpair[:CSE] = gam[HI] * gam[HJ]
    bpair[:CSE] = bet[HI] * bet[HJ]
    ghi, ghj = _make_consts()
    return dict(
        w2T=np.ascontiguousarray(W2.T),
        w2rows=np.ascontiguousarray(W2),
        eT=np.ascontiguousarray(E64.astype(np.float32).T),
        f2=np.broadcast_to(f64.astype(np.float32), (SPC, C1)).copy(),
        b2vec=b2_64.astype(np.float32),
        gpairp=gpair,
        bpairp=bpair,
        g_hi=ghi,
        g_hj=ghj,
        ident32=np.eye(CS, dtype=np.float32),
        ident128=np.eye(P, dtype=np.float32),
    )


def make_in_maps(inputs):
    x = np.asarray(inputs["x"], np.float32).reshape(B, C1, HWD)
    shared = make_shared_inputs(
        inputs["fc_w"], inputs["fc_b"], inputs["bn_gamma"], inputs["bn_beta"],
        inputs["bn_mean"], inputs["bn_var"], inputs["eva_w"], inputs["eva_b"],
    )
    return [
        dict(shared, xs=np.ascontiguousarray(x[i * SPC : (i + 1) * SPC]))
        for i in range(NCORES)
    ]


def kernel(x, fc_w, fc_b, bn_gamma, bn_beta, bn_mean, bn_var, eva_w, eva_b):
    in_maps = make_in_maps(dict(
        x=x, fc_w=fc_w, fc_b=fc_b, bn_gamma=bn_gamma, bn_beta=bn_beta,
        bn_mean=bn_mean, bn_var=bn_var, eva_w=eva_w, eva_b=eva_b,
    ))
    nc = _get_program()
    res = run_bass_kernel_spmd(nc, in_maps, list(range(NCORES))).results
    out = np.empty((B, C1 + CSE, HWD), np.float32)
    for i in range(NCORES):
        for s in range(SPC):
            out[i * SPC + s] = res[i][f"out{s}"]
    return out.reshape(B, C1 + CSE, 32, 32)


# revision 16
# speedup vs baseline: 1.3492x; 1.3429x over previous
"""Trainium2 Bass kernel for nn_HadamardExpansionV2 (topk_masking).

Sharding: data-parallel over batch B=16 across 8 cores (2 samples/core);
weights replicated. CrossHadaNorm batch stats via AllReduce of per-channel
sum/sumsq.

Host precompute folds BN into the conv: W2 = diag(s) @ fc_w, b2 = (fc_b -
bn_mean)*s + bn_beta with s = gamma/sqrt(var+eps), so y = W2 @ x + b2.
Logits fold: E = (eva_w @ W2)/1024, f = eva_w @ b2 + eva_b, so
logits = E @ sum_hw(x) + f (computed exact fp32 on device: min top-32
boundary gap is ~1e-4, so the logits path cannot use f32r).

Per-core schedule:
  - x/weight chunks load on sync-DGE; E^T/f/consts load via scalar-DGE in
    parallel; per-chunk tiles so consumers start as chunks land.
  - PE: conv m0 -> logits -> conv m1..m3 (covers the top-k wait) ->
    Wsel transposes -> selconv -> hadamard pa/pb.
  - DVE: xbar row-sums early, top-k rounds, prod multiplies, s1 reduces,
    post-AllReduce affine.  ACT: conv epilogues, pa copies, Square+accum
    for s2.  A dummy AllReduce at t=0 warms the collective stream.
  - AllReduce of [s1|s2] -> affine -> streamed prod writes.
"""

import os
import sys

import numpy as np

for _p in ("/opt/trn_rl_repo", os.path.expanduser("~/.axon_site/_ro/trn_rl_repo")):
    if os.path.isdir(_p) and _p not in sys.path:
        sys.path.insert(0, _p)

import concourse.bass as bass
import concourse.bass_isa as bass_isa
import concourse.mybir as mybir
import concourse.tile as tile
from concourse import bacc
from concourse.bass_utils import run_bass_kernel_spmd

C1 = 512
CS = 32
CSE = 496  # 32*31/2
HWD = 1024  # H*W
B = 16
NCORES = 8
SPC = B // NCORES  # samples per core
P = 128
KC = C1 // P  # 4 contraction chunks
MC = C1 // P  # 4 output-channel chunks
NF = 512  # matmul free dim (PSUM bank)
NNC = HWD // NF  # 2 free chunks
EPS = 1e-5
NTOT = float(B * HWD)

HI, HJ = np.triu_indices(CS, k=1)

F32 = mybir.dt.float32
F32R = mybir.dt.float32r
U32 = mybir.dt.uint32

# M-chunking of the 496 expanded channels: 128,128,128,112
EXP_M = [(0, 128), (128, 128), (256, 128), (384, 112)]

USE_TTR = os.environ.get("USE_TTR", "0") == "1"
WARM_CC = os.environ.get("WARM_CC", "1") == "1"


def build_program():
    nc = bacc.Bacc(
        "TRN2",
        target_bir_lowering=False,
        debug=False,
        num_devices=NCORES,
    )

    # ---------------- I/O ----------------
    xs = nc.dram_tensor("xs", [SPC, C1, HWD], F32R, kind="ExternalInput")
    w2T = nc.dram_tensor("w2T", [C1, C1], F32R, kind="ExternalInput")  # [c, o]
    w2rows = nc.dram_tensor("w2rows", [C1, C1], F32, kind="ExternalInput")  # [o, c]
    eT = nc.dram_tensor("eT", [C1, C1], F32, kind="ExternalInput")  # [c, o']
    fcat = nc.dram_tensor("fcat", [1, SPC * C1], F32, kind="ExternalInput")
    rowmask = nc.dram_tensor("rowmask", [SPC, C1], mybir.dt.int32, kind="ExternalInput")
    b2vec = nc.dram_tensor("b2vec", [C1], F32, kind="ExternalInput")
    gpairp = nc.dram_tensor("gpairp", [C1], F32, kind="ExternalInput")
    bpairp = nc.dram_tensor("bpairp", [C1], F32, kind="ExternalInput")
    g_hi = nc.dram_tensor("g_hi", [CS, CSE], F32R, kind="ExternalInput")
    g_hj = nc.dram_tensor("g_hj", [CS, CSE], F32R, kind="ExternalInput")
    ident32 = nc.dram_tensor("ident32", [CS, CS], F32, kind="ExternalInput")

    outs = [
        nc.dram_tensor(f"out{s}", [C1 + CSE, HWD], F32, kind="ExternalOutput")
        for s in range(SPC)
    ]
    dbg_logit = nc.dram_tensor("dbg_logit", [SPC, C1], F32, kind="ExternalOutput")
    dbg_idx = nc.dram_tensor("dbg_idx", [SPC, CS], U32, kind="ExternalOutput")

    with tile.TileContext(nc) as tc:
        with (
            tc.tile_pool(name="const", bufs=1) as cpool,
            tc.tile_pool(name="xp", bufs=1) as xpool,
            tc.tile_pool(name="prodp", bufs=1) as prodpool,
            tc.tile_pool(name="yp", bufs=8) as ypool,
            tc.tile_pool(name="acp", bufs=4) as apool,
            tc.tile_pool(name="small", bufs=3) as spool,
            tc.tile_pool(name="psA", bufs=4, space="PSUM") as psA,
            tc.tile_pool(name="psB", bufs=4, space="PSUM") as psB,
            tc.tile_pool(name="dram", bufs=1, space="DRAM") as dpool,
        ):
            # ---- persistent SBUF tiles (per-chunk for dep granularity) ----
            w_k = [cpool.tile([P, C1], F32R, tag=f"w{k}", name=f"w{k}") for k in range(KC)]
            x_k = [
                [xpool.tile([P, HWD], F32R, tag=f"x{s}_{k}", name=f"x{s}_{k}") for k in range(KC)]
                for s in range(SPC)
            ]
            e_k = [cpool.tile([P, C1], F32, tag=f"e{k}", name=f"e{k}") for k in range(KC)]
            fcat_sb = cpool.tile([1, SPC * C1], F32, tag="fcat", name="fcat")
            rowmask_sb = cpool.tile([SPC, C1], mybir.dt.int32, tag="rmask", name="rmask")
            b2v = cpool.tile([P, MC], F32, tag="b2v", name="b2v")
            gpair_v = cpool.tile([P, MC], F32, tag="gpv", name="gpv")
            bpair_v = cpool.tile([P, MC], F32, tag="bpv", name="bpv")
            i32_sb = cpool.tile([CS, CS], F32, tag="i32", name="i32")
            ghi_sb = cpool.tile([CS, CSE], F32R, tag="ghi", name="ghi")
            ghj_sb = cpool.tile([CS, CSE], F32R, tag="ghj", name="ghj")
            xbar2 = cpool.tile([P, KC, SPC], F32, tag="xbar", name="xbar")
            stats = cpool.tile([P, 2 * MC], F32, tag="stats", name="stats")
            s1p = cpool.tile([P, MC, NNC * SPC], F32, tag="s1p", name="s1p")
            s2p = cpool.tile([P, MC, NNC * SPC], F32, tag="s2p", name="s2p")
            gstats = cpool.tile([P, 2 * MC], F32, tag="gstats", name="gstats")
            eps_col = cpool.tile([P, 1], F32, tag="eps", name="eps")
            prod_sb = [
                prodpool.tile([P, MC, HWD], F32, tag=f"prod{s}", name=f"prod{s}")
                for s in range(SPC)
            ]

            # ---- collective warm-up: trigger-only dummy AllReduce on stream 0
            # (reads uninitialized DRAM; result unused; no data dependencies)
            if WARM_CC:
                ccw_in = dpool.tile([1, 8], F32)
                ccw_out = dpool.tile([1, 8], F32)
                nc.gpsimd.collective_compute(
                    "AllReduce",
                    mybir.AluOpType.add,
                    replica_groups=[list(range(NCORES))],
                    ins=[ccw_in.opt()],
                    outs=[ccw_out.opt()],
                )

            # ---- input DMAs: conv weights + x interleaved per k (sync DGE) ----
            for k in range(KC):
                nc.sync.dma_start(w_k[k][:], w2T.ap()[k * P : (k + 1) * P, :])
                nc.sync.dma_start(x_k[0][k][:], xs.ap()[0][k * P : (k + 1) * P, :])
            for k in range(KC):
                nc.scalar.dma_start(x_k[1][k][:], xs.ap()[1][k * P : (k + 1) * P, :])
            # consts via scalar DGE (parallel queue)
            for k in range(KC):
                nc.scalar.dma_start(e_k[k][:], eT.ap()[k * P : (k + 1) * P, :])
            nc.scalar.dma_start(fcat_sb[:], fcat.ap())
            nc.scalar.dma_start(rowmask_sb[:], rowmask.ap())
            nc.scalar.dma_start(b2v[:], b2vec.ap().rearrange("(m p) -> p m", p=P))
            nc.scalar.dma_start(gpair_v[:], gpairp.ap().rearrange("(m p) -> p m", p=P))
            nc.scalar.dma_start(bpair_v[:], bpairp.ap().rearrange("(m p) -> p m", p=P))
            nc.scalar.dma_start(i32_sb[:], ident32.ap())
            nc.scalar.dma_start(ghi_sb[:], g_hi.ap())
            nc.scalar.dma_start(ghj_sb[:], g_hj.ap())

            # ---- Pool: memsets;  DVE: early xbar row-sums ----
            nc.gpsimd.memset(stats[:], 0.0)
            nc.gpsimd.memset(s1p[:], 0.0)
            nc.gpsimd.memset(s2p[:], 0.0)
            nc.gpsimd.memset(eps_col[:], EPS)
            sqwarm = cpool.tile([P, 1], F32, tag="sqw", name="sqw")
            nc.scalar.activation(
                sqwarm[:], eps_col[:], mybir.ActivationFunctionType.Sqrt,
                bias=eps_col[:, 0:1],
            )
            # two-stage tree reduce: plain sequential f32 summation of 1024
            # values carries ~1e-4 error on sums ~30, which maps through E to
            # ~3e-6 logit error and flips a 2.45e-6 top-32 gap in sample 14.
            xt1 = cpool.tile([P, 32], F32, tag="xt1", name="xt1")
            for k in range(KC):
                for s in range(SPC):
                    nc.vector.tensor_reduce(
                        xt1[:],
                        x_k[s][k].bitcast(F32).rearrange("p (a b) -> p a b", b=32),
                        axis=mybir.AxisListType.X,
                        op=mybir.AluOpType.add,
                    )
                    nc.vector.tensor_reduce(
                        xbar2[:, k, s : s + 1],
                        xt1[:],
                        axis=mybir.AxisListType.X,
                        op=mybir.AluOpType.add,
                    )

            # ---- conv helper: one m-chunk, both samples ----
            def conv_mgroup(m):
                pts = {}
                for s in range(SPC):
                    for n in range(NNC):
                        pts[(s, n)] = psA.tile([P, NF], F32, tag="cv", name=f"cv{m}_{s}_{n}")
                for k in range(KC):
                    for s in range(SPC):
                        for n in range(NNC):
                            nc.tensor.matmul(
                                pts[(s, n)][:],
                                lhsT=w_k[k][:, m * P : (m + 1) * P],
                                rhs=x_k[s][k][:, n * NF : (n + 1) * NF],
                                start=(k == 0),
                                stop=(k == KC - 1),
                            )
                for s in range(SPC):
                    ych = ypool.tile([P, HWD], F32, tag="y", name=f"y{m}_{s}")
                    for n in range(NNC):
                        nc.scalar.activation(
                            ych[:, n * NF : (n + 1) * NF],
                            pts[(s, n)][:],
                            mybir.ActivationFunctionType.Identity,
                            bias=b2v[:, m : m + 1],
                            scale=1.0,
                        )
                    nc.sync.dma_start(
                        outs[s].ap()[m * P : (m + 1) * P, :], ych[:]
                    )

            conv_mgroup(0)

            # ---- logits: exact f32 on DVE + gpsimd partition all-reduce.
            # The PE's fp32 matmul path is not bit-accurate enough: the top-32
            # of sample 14 contains a 2.4e-6 logit gap that it reorders.
            # acc_all[c, s*512+o] = E^T[c, o] * xbar[c, s]; f folded into
            # partition 0; one all-reduce over partitions; select() assembles
            # the [2, 512] row layout (engine APs must start at partition 0).
            acc_all = cpool.tile([P, SPC * C1], F32, tag="lacc", name="lacc")
            for s in range(SPC):
                nc.vector.tensor_scalar_mul(
                    acc_all[:, s * C1 : (s + 1) * C1], e_k[0][:],
                    xbar2[:, 0, s : s + 1],
                )
                for k in range(1, KC):
                    nc.vector.scalar_tensor_tensor(
                        out=acc_all[:, s * C1 : (s + 1) * C1],
                        in0=e_k[k][:],
                        scalar=xbar2[:, k, s : s + 1],
                        in1=acc_all[:, s * C1 : (s + 1) * C1],
                        op0=mybir.AluOpType.mult,
                        op1=mybir.AluOpType.add,
                    )
            nc.vector.tensor_tensor(
                acc_all[0:1, :], acc_all[0:1, :], fcat_sb[:],
                op=mybir.AluOpType.add,
            )
            lred = cpool.tile([P, SPC * C1], F32, tag="lred", name="lred")
            nc.gpsimd.partition_all_reduce(
                lred[:], acc_all[:], channels=P, reduce_op=bass_isa.ReduceOp.add
            )
            logit = cpool.tile([SPC, C1], F32, tag="logit", name="logit")
            nc.vector.select(
                out=logit[:],
                mask=rowmask_sb[:],
                on_true=lred[0:SPC, C1 : C1 + C1],
                on_false=lred[0:SPC, 0:C1],
            )

            # ---- top-32 (DVE) + idx roundtrip — emitted early so the idx
            # DMAs sit ahead of later y-write triggers on the sync queue ----
            idx_all = cpool.tile([SPC, CS], U32, tag="idx", name="idx")
            for r in range(4):
                mx8 = spool.tile([SPC, 8], F32, tag="mx8", name=f"mx8_{r}")
                nc.vector.max(out=mx8[:], in_=logit[:])
                nc.vector.max_index(
                    out=idx_all[:, r * 8 : (r + 1) * 8], in_max=mx8[:], in_values=logit[:]
                )
                if r < 3:
                    nc.vector.match_replace(
                        out=logit[:], in_to_replace=mx8[:], in_values=logit[:],
                        imm_value=-1e30,
                    )
            nc.sync.dma_start(dbg_logit.ap(), logit[:])
            nc.sync.dma_start(dbg_idx.ap(), idx_all[:])
            idx_dram = dpool.tile([SPC, CS], U32)
            nc.sync.dma_start(idx_dram[:], idx_all[:])
            idx_col = [None] * SPC
            for s in range(SPC):
                idx_col[s] = cpool.tile([CS, 1], U32, tag=f"idxc{s}", name=f"idxc{s}")
                nc.sync.dma_start(
                    idx_col[s][:],
                    idx_dram[s].rearrange("(p one) -> p one", one=1),
                )
            # gathers (gpsimd stream; fire when idx lands)
            wsel = [None] * SPC
            b2sel = [None] * SPC
            for s in range(SPC):
                wsel[s] = cpool.tile([CS, C1], F32, tag=f"wsel{s}", name=f"wsel{s}")
                nc.gpsimd.indirect_dma_start(
                    out=wsel[s][:],
                    out_offset=None,
                    in_=w2rows.ap()[0:C1, :],
                    in_offset=bass.IndirectOffsetOnAxis(ap=idx_col[s][:, :1], axis=0),
                )
                b2sel[s] = cpool.tile([CS, 1], F32, tag=f"b2s{s}", name=f"b2s{s}")
                nc.gpsimd.indirect_dma_start(
                    out=b2sel[s][:],
                    out_offset=None,
                    in_=b2vec.ap().rearrange("(c one) -> c one", one=1),
                    in_offset=bass.IndirectOffsetOnAxis(ap=idx_col[s][:, :1], axis=0),
                )

            # ---- conv m1..m3 (PE covers the top-k wait) ----
            conv_mgroup(1)
            conv_mgroup(2)
            conv_mgroup(3)

            # ---- selconv: xsel = W2[idx] @ x + b2[idx] ----
            xsel = [None] * SPC
            for s in range(SPC):
                ptr = psB.tile([P, NF], F32, tag="pa", name=f"ptr{s}")
                for k in range(KC):
                    nc.tensor.transpose(
                        ptr[:, k * CS : (k + 1) * CS],
                        wsel[s][:, k * P : (k + 1) * P],
                        i32_sb[:],
                    )
                wselT = cpool.tile([P, KC, CS], F32R, tag=f"wT{s}", name=f"wselT{s}")
                nc.scalar.activation(
                    wselT[:], ptr[:, 0 : KC * CS], mybir.ActivationFunctionType.Copy
                )
                xsel[s] = cpool.tile([CS, HWD], F32R, tag=f"xsel{s}", name=f"xsel{s}")
                for n in range(NNC):
                    psel = psA.tile([P, NF], F32, tag="cv", name=f"psel{s}_{n}")
                    for k in range(KC):
                        nc.tensor.matmul(
                            psel[:CS, :],
                            lhsT=wselT[:, k, :],
                            rhs=x_k[s][k][:, n * NF : (n + 1) * NF],
                            start=(k == 0),
                            stop=(k == KC - 1),
                        )
                    nc.scalar.activation(
                        xsel[s][:, n * NF : (n + 1) * NF],
                        psel[:CS, :],
                        mybir.ActivationFunctionType.Identity,
                        bias=b2sel[s][:, 0:1],
                        scale=1.0,
                    )

            # ---- hadamard expansion + batch stats ----
            for m, (mo, mw) in enumerate(EXP_M):
                pas = {}
                for s in range(SPC):
                    for n in range(NNC):
                        pas[(s, n)] = psB.tile([P, NF], F32, tag="pa", name=f"pa{m}_{s}_{n}")
                        nc.tensor.matmul(
                            pas[(s, n)][:mw, :],
                            lhsT=ghi_sb[:, mo : mo + mw],
                            rhs=xsel[s][:, n * NF : (n + 1) * NF],
                            start=True,
                            stop=True,
                        )
                pbs = {}
                for s in range(SPC):
                    for n in range(NNC):
                        pbs[(s, n)] = psA.tile([P, NF], F32, tag="cv", name=f"pb{m}_{s}_{n}")
                        nc.tensor.matmul(
                            pbs[(s, n)][:mw, :],
                            lhsT=ghj_sb[:, mo : mo + mw],
                            rhs=xsel[s][:, n * NF : (n + 1) * NF],
                            start=True,
                            stop=True,
                        )
                for s in range(SPC):
                    for n in range(NNC):
                        a_sb = apool.tile([P, NF], F32, tag="ac", name=f"ac{m}_{s}_{n}")
                        nc.scalar.activation(
                            a_sb[:mw, :], pas[(s, n)][:mw, :],
                            mybir.ActivationFunctionType.Copy,
                        )
                        pchunk = prod_sb[s][:, m, n * NF : (n + 1) * NF]
                        if USE_TTR:
                            nc.vector.tensor_tensor_reduce(
                                out=pchunk[:mw],
                                in0=a_sb[:mw, :],
                                in1=pbs[(s, n)][:mw, :],
                                scale=1.0,
                                scalar=stats[:mw, m : m + 1],
                                op0=mybir.AluOpType.mult,
                                op1=mybir.AluOpType.add,
                                accum_out=stats[:mw, m : m + 1],
                            )
                        else:
                            nc.vector.tensor_tensor(
                                pchunk[:mw], a_sb[:mw, :], pbs[(s, n)][:mw, :],
                                op=mybir.AluOpType.mult,
                            )
                            nc.vector.tensor_reduce(
                                s1p[:mw, m, n * SPC + s : n * SPC + s + 1],
                                pchunk[:mw],
                                axis=mybir.AxisListType.X,
                                op=mybir.AluOpType.add,
                            )
                        sq = apool.tile([P, NF], F32, tag="sq", name=f"sq{m}_{s}_{n}")
                        nc.scalar.activation(
                            sq[:mw, :],
                            pchunk[:mw],
                            mybir.ActivationFunctionType.Square,
                            accum_out=s2p[:mw, m, n * SPC + s : n * SPC + s + 1],
                        )

            # slots -> stats cols
            if not USE_TTR:
                nc.vector.tensor_reduce(
                    stats[:, 0:MC], s1p[:], axis=mybir.AxisListType.X,
                    op=mybir.AluOpType.add,
                )
            nc.vector.tensor_reduce(
                stats[:, MC : 2 * MC], s2p[:], axis=mybir.AxisListType.X,
                op=mybir.AluOpType.add,
            )

            # ---- cross-core AllReduce of [s1|s2] ----
            cc_in = dpool.tile([P, 2 * MC], F32)
            cc_out = dpool.tile([P, 2 * MC], F32)
            nc.sync.dma_start(cc_in[:], stats[:])
            nc.gpsimd.collective_compute(
                "AllReduce",
                mybir.AluOpType.add,
                replica_groups=[list(range(NCORES))],
                ins=[cc_in.opt()],
                outs=[cc_out.opt()],
            )
            nc.sync.dma_start(gstats[:], cc_out[:])

            # mean/var -> alpha = rstd*gpair ; beta2 = bpair - mean*alpha
            meanc = cpool.tile([P, MC], F32, tag="meanc", name="meanc")
            nc.vector.tensor_scalar_mul(meanc[:], gstats[:, 0:MC], 1.0 / NTOT)
            varc = cpool.tile([P, MC], F32, tag="varc", name="varc")
            nc.vector.tensor_scalar_mul(varc[:], gstats[:, MC : 2 * MC], 1.0 / NTOT)
            msq = cpool.tile([P, MC], F32, tag="msq", name="msq")
            nc.vector.tensor_mul(msq[:], meanc[:], meanc[:])
            nc.vector.tensor_sub(varc[:], varc[:], msq[:])
            nc.scalar.activation(
                varc[:], varc[:], mybir.ActivationFunctionType.Sqrt,
                bias=eps_col[:, 0:1],
            )
            rstd = cpool.tile([P, MC], F32, tag="rstd", name="rstd")
            nc.vector.reciprocal(rstd[:], varc[:])
            alpha = cpool.tile([P, MC], F32, tag="alpha", name="alpha")
            nc.vector.tensor_mul(alpha[:], rstd[:], gpair_v[:])
            beta2 = cpool.tile([P, MC], F32, tag="beta2", name="beta2")
            nc.vector.tensor_mul(beta2[:], meanc[:], alpha[:])
            nc.vector.tensor_sub(beta2[:], bpair_v[:], beta2[:])

            # ---- normalize + write prod rows (per (s, m), full 1024 cols);
            # alternate engines/queues to halve the post-AllReduce tail ----
            for s in range(SPC):
                for m, (mo, mw) in enumerate(EXP_M):
                    pch = prod_sb[s][:, m, :]
                    if (s * MC + m) % 2 == 0:
                        nc.vector.tensor_scalar(
                            pch[:mw],
                            pch[:mw],
                            alpha[:mw, m : m + 1],
                            beta2[:mw, m : m + 1],
                            op0=mybir.AluOpType.mult,
                            op1=mybir.AluOpType.add,
                        )
                        nc.sync.dma_start(
                            outs[s].ap()[C1 + mo : C1 + mo + mw, :], pch[:mw]
                        )
                    else:
                        nc.scalar.activation(
                            pch[:mw],
                            pch[:mw],
                            mybir.ActivationFunctionType.Identity,
                            bias=beta2[:mw, m : m + 1],
                            scale=alpha[:mw, m : m + 1],
                        )
                        nc.scalar.dma_start(
                            outs[s].ap()[C1 + mo : C1 + mo + mw, :], pch[:mw]
                        )

    nc.compile()
    return nc


_NC_CACHE = {}
LAST_RES = None


def _get_program():
    if "nc" not in _NC_CACHE:
        _NC_CACHE["nc"] = build_program()
    return _NC_CACHE["nc"]


def _make_consts():
    ghi = np.zeros((CS, CSE), np.float32)
    ghj = np.zeros((CS, CSE), np.float32)
    ghi[HI, np.arange(CSE)] = 1.0
    ghj[HJ, np.arange(CSE)] = 1.0
    return ghi, ghj


def make_shared_inputs(fc_w, fc_b, bn_gamma, bn_beta, bn_mean, bn_var, eva_w, eva_b):
    g64 = np.asarray(bn_gamma, np.float64)
    s64 = g64 / np.sqrt(np.asarray(bn_var, np.float64) + EPS)
    W2_64 = s64[:, None] * np.asarray(fc_w, np.float64)
    b2_64 = (np.asarray(fc_b, np.float64) - np.asarray(bn_mean, np.float64)) * s64 \
        + np.asarray(bn_beta, np.float64)
    E64 = (np.asarray(eva_w, np.float64) @ W2_64) / float(HWD)
    f64 = np.asarray(eva_w, np.float64) @ b2_64 + np.asarray(eva_b, np.float64)

    W2 = W2_64.astype(np.float32)
    gam = np.asarray(bn_gamma, np.float32)
    bet = np.asarray(bn_beta, np.float32)
    gpair = np.zeros(C1, np.float32)
    bpair = np.zeros(C1, np.float32)
    gpair[:CSE] = gam[HI] * gam[HJ]
    bpair[:CSE] = bet[HI] * bet[HJ]
    ghi, ghj = _make_consts()
    return dict(
        w2T=np.ascontiguousarray(W2.T),
        w2rows=np.ascontiguousarray(W2),
        eT=np.ascontiguousarray(E64.astype(np.float32).T),
        fcat=np.tile(f64.astype(np.float32), SPC)[None, :],
        rowmask=np.repeat(np.arange(SPC, dtype=np.int32)[:, None], C1, axis=1),
        b2vec=b2_64.astype(np.float32),
        gpairp=gpair,
        bpairp=bpair,
        g_hi=ghi,
        g_hj=ghj,
        ident32=np.eye(CS, dtype=np.float32),
    )


def make_in_maps(inputs):
    x = np.asarray(inputs["x"], np.float32).reshape(B, C1, HWD)
    shared = make_shared_inputs(
        inputs["fc_w"], inputs["fc_b"], inputs["bn_gamma"], inputs["bn_beta"],
        inputs["bn_mean"], inputs["bn_var"], inputs["eva_w"], inputs["eva_b"],
    )
    return [
        dict(shared, xs=np.ascontiguousarray(x[i * SPC : (i + 1) * SPC]))
        for i in range(NCORES)
    ]


def kernel(x, fc_w, fc_b, bn_gamma, bn_beta, bn_mean, bn_var, eva_w, eva_b):
    in_maps = make_in_maps(dict(
        x=x, fc_w=fc_w, fc_b=fc_b, bn_gamma=bn_gamma, bn_beta=bn_beta,
        bn_mean=bn_mean, bn_var=bn_var, eva_w=eva_w, eva_b=eva_b,
    ))
    nc = _get_program()
    res = run_bass_kernel_spmd(nc, in_maps, list(range(NCORES))).results
    global LAST_RES
    LAST_RES = res
    out = np.empty((B, C1 + CSE, HWD), np.float32)
    for i in range(NCORES):
        for s in range(SPC):
            out[i * SPC + s] = res[i][f"out{s}"]
    return out.reshape(B, C1 + CSE, 32, 32)
